# revision 18
# baseline (speedup 1.0000x reference)
"""Trainium2 Bass kernel for DynamicCondLinear (MoE-routing style).

Math: condition batch is 1, so softmax routing weights alpha (K=8) are shared
by all 32 samples; out = sum_k alpha_k * (x @ W_k^T) + sum_k alpha_k * b_k.

Sharding: tensor-parallel over OUT channels (2048 / 8 cores = 256 per core).
Each core streams its 16 MiB weight shard from HBM once (memory roofline),
computes alpha redundantly (w1 replicated), and does the whole contraction as
accumulating float32r matmuls in a single PSUM tile.

Host-side prep is layout-only (transpose/reshape for DMA-friendly tiling).
"""

import os
import sys

import numpy as np

if "/opt/trn_rl_repo" not in sys.path:
    sys.path.insert(0, "/opt/trn_rl_repo")

import concourse.bacc as bacc
import concourse.mybir as mybir
import concourse.tile as tile
from concourse.bass_utils import run_bass_kernel_spmd

B, IN, OUT, K, H = 32, 2048, 2048, 8, 512
NCORES = 8
OC = OUT // NCORES  # 256 out channels per core
JT = IN // 128      # 16 contraction tiles
HT = H // 128       # 4 hidden tiles
HS = H // NCORES    # 64 hidden units per core when w1 is sharded
# sharding w1 + AllReduce of partial scores measured 190us vs 63us
# replicated: the 8-core collective costs ~120us here, far more than
# the 5us of DMA it saves. Keep w1 replicated (bf16 halves it anyway).
SHARD_W1 = os.environ.get("KERNEL_SHARD_W1", "0") == "1"

F32 = mybir.dt.float32
F32R = mybir.dt.float32r
BF16 = mybir.dt.bfloat16
FP16 = mybir.dt.float16
if os.environ.get("KERNEL_NO_F32R", "0") == "1":
    F32R = mybir.dt.float32  # debug toggle: run everything in plain fp32
# Main path (wt/xk) dtype: fp16 has the same 10-bit mantissa as f32r
# (TF32) -> identical output error (measured 2.95e-4 both ways) at half
# the HBM bytes. f32r/f32 available as fallbacks via env.
_WT = os.environ.get("KERNEL_WT_DTYPE", "fp16")
F32R_MAIN = {"fp16": FP16, "f32r": F32R, "f32": F32}[_WT]

_CACHE = {}

# test.py reads this after calling kernel() to get profiling info
LAST_RESULTS = None


def _build_module():
    """Build the (SPMD-identical) Bass program once."""
    nc = bacc.Bacc("TRN2", target_bir_lowering=False, debug=False,
                   num_devices=NCORES)

    # --- DRAM I/O (per-core data differs, program is identical) ---
    # Pure-matmul operands are declared float32r end-to-end (same bits as
    # fp32; PE runs them at 1 cyc/row instead of 4).
    wt_d = nc.dram_tensor("wt", (K, 128, JT * OC), F32R_MAIN, kind="ExternalInput")
    xt_d = nc.dram_tensor("xt", (128, JT * B), F32, kind="ExternalInput")
    HL = HS if SHARD_W1 else H  # local hidden width
    ct_d = nc.dram_tensor("ct", (128, JT), BF16, kind="ExternalInput")
    w1t_d = nc.dram_tensor("w1t", (128, JT * HL), BF16, kind="ExternalInput")
    if SHARD_W1:
        w2t_d = nc.dram_tensor("w2t", (HS, K), F32, kind="ExternalInput")
    else:
        w2t_d = nc.dram_tensor("w2t", (128, HT * K), F32, kind="ExternalInput")
    b1r_d = nc.dram_tensor("b1r", (1, HL), F32, kind="ExternalInput")
    b2r_d = nc.dram_tensor("b2r", (1, K), F32, kind="ExternalInput")
    kb_d = nc.dram_tensor("kb", (K, OC), F32, kind="ExternalInput")
    y_d = nc.dram_tensor("y", (B, OC), F32, kind="ExternalOutput")
    # warmup sink: consumed so bacc's DCE keeps the PE warm-up matmuls
    ysink_d = nc.dram_tensor("ysink", (1, 1), F32, kind="ExternalOutput")

    with tile.TileContext(nc) as tc:
        with (
            tc.tile_pool(name="cpool", bufs=1) as cpool,
            tc.tile_pool(name="wpool", bufs=16) as wpool,
            tc.tile_pool(name="ppool", bufs=1, space="PSUM") as ppool,
        ):
            # --- weight stream first: 16 resident 1 MiB slabs on the sync
            # HWDGE ring (the critical stream; small loads go on the scalar
            # ring so they don't serialize in front of it) ---
            # slab plan: (k, j_start, n_j) -- one slab per expert (1 MiB fp16)
            SLAB_PLAN = [(k, 0, JT) for k in range(K)]
            slabs = []
            for (k, j0, nj) in SLAB_PLAN:
                wt_slab = wpool.tile((128, JT * OC), F32R_MAIN, tag="wt_slab",
                                     bufs=len(SLAB_PLAN))
                nc.sync.dma_start(
                    wt_slab[:, :nj * OC],
                    wt_d.ap()[k][:, j0 * OC:(j0 + nj) * OC])
                slabs.append(wt_slab)

            # --- PE warm-up: dependency-free dummy matmuls keep the HAM
            # clock gate at 2.4 GHz while DMAs land and the alpha chain
            # runs on DVE/ACT. Results go to a scratch PSUM bank. ---
            dum_a = cpool.tile((128, B), BF16)
            nc.gpsimd.memset(dum_a[:], 0.0)
            dum_b = cpool.tile((128, OC), BF16)
            nc.gpsimd.memset(dum_b[:], 0.0)
            dum_psum = ppool.tile((B, OC), F32)

            def warmup(n):
                for _ in range(n):
                    nc.tensor.matmul(dum_psum[:], dum_a[:], dum_b[:],
                                     start=True, stop=True)

            dum_sink = cpool.tile((1, 1), F32)

            # --- constant / small loads ---
            ct_sb = cpool.tile((128, JT), BF16)
            nc.scalar.dma_start(ct_sb[:], ct_d.ap())
            w1t_sb = cpool.tile((128, JT, HL), BF16)
            w1t_ap = w1t_d.ap().rearrange("p (t h) -> p t h", t=JT)
            for cch in range(4):
                tsl = slice(cch * (JT // 4), (cch + 1) * (JT // 4))
                nc.scalar.dma_start(w1t_sb[:, tsl, :], w1t_ap[:, tsl, :])
            if SHARD_W1:
                w2t_sb = cpool.tile((HS, K), F32)
            else:
                w2t_sb = cpool.tile((128, HT, K), F32)
            nc.scalar.dma_start(w2t_sb[:], w2t_d.ap())
            b1r_sb = cpool.tile((1, HL), F32)
            nc.scalar.dma_start(b1r_sb[:], b1r_d.ap())
            b2r_sb = cpool.tile((1, K), F32)
            nc.scalar.dma_start(b2r_sb[:], b2r_d.ap())
            kb_sb = cpool.tile((K, OC), F32)
            nc.scalar.dma_start(kb_sb[:], kb_d.ap())
            xt_sb = cpool.tile((128, JT * B), F32)
            nc.scalar.dma_start(xt_sb[:], xt_d.ap())

            ones1 = cpool.tile((1, 1), F32)
            nc.gpsimd.memset(ones1[:], 1.0)
            ones_b = cpool.tile((1, B), BF16)
            nc.gpsimd.memset(ones_b[:], 1.0)
            ones_p = cpool.tile((1, 128), F32)
            nc.gpsimd.memset(ones_p[:], 1.0)

            warmup(int(os.environ.get("KERNEL_WARMUP1", "24")))

            # --- alpha MLP: h = relu(cond @ w1 + b1), hidden sharded
            # over cores when SHARD_W1 (each core computes HS=64 units and
            # its partial scores; AllReduce sums the (1,8) partials) ---
            psum_h = ppool.tile((1, HL), F32)
            for t in range(JT):
                nc.tensor.matmul(
                    psum_h[:],
                    ct_sb[:, t:t + 1],
                    w1t_sb[:, t, :],
                    start=(t == 0), stop=(t == JT - 1),
                )
            h_tmp = cpool.tile((1, HL), F32)
            nc.vector.tensor_add(h_tmp[:], psum_h[:], b1r_sb[:])
            h_sb = cpool.tile((1, HL), F32)
            nc.vector.tensor_relu(h_sb[:], h_tmp[:])

            # transpose h (1,HL) -> hT (HL? x 1) via tiny matmuls vs ones
            NQ = (HL + 127) // 128
            QW = min(HL, 128)
            psum_ht = ppool.tile((QW, NQ), F32)
            for q in range(NQ):
                nc.tensor.matmul(
                    psum_ht[:, q:q + 1],
                    h_sb[:, q * QW:(q + 1) * QW],
                    ones1[:],
                    start=True, stop=True,
                )
            ht_sb = cpool.tile((QW, NQ), F32)
            nc.vector.tensor_copy(ht_sb[:], psum_ht[:])

            # scores row (1, 8) = sum_q hT[:,q].T @ w2t rows
            psum_s = ppool.tile((1, K), F32)
            if SHARD_W1:
                nc.tensor.matmul(psum_s[:], ht_sb[:, 0:1], w2t_sb[:],
                                 start=True, stop=True)
            else:
                for q in range(NQ):
                    nc.tensor.matmul(
                        psum_s[:],
                        ht_sb[:, q:q + 1],
                        w2t_sb[:, q, :],
                        start=(q == 0), stop=(q == NQ - 1),
                    )

            if SHARD_W1:
                # AllReduce the (1, 8) partial scores across the 8 cores
                with tc.tile_pool(name="dpool", bufs=1, space="DRAM") as dpool:
                    sc_in = dpool.tile((1, K), F32)
                    sc_out = dpool.tile((1, K), F32)
                    sp_sb = cpool.tile((1, K), F32)
                    nc.vector.tensor_copy(sp_sb[:], psum_s[:])
                    nc.gpsimd.dma_start(sc_in[:], sp_sb[:])
                    nc.gpsimd.collective_compute(
                        "AllReduce",
                        mybir.AluOpType.add,
                        replica_groups=[list(range(NCORES))],
                        ins=[sc_in.opt()],
                        outs=[sc_out.opt()],
                    )
                    sall_sb = cpool.tile((1, K), F32)
                    nc.gpsimd.dma_start(sall_sb[:], sc_out[:])

            warmup(int(os.environ.get("KERNEL_WARMUP2", "16")))

            # softmax over the 8 scores (+ b2)
            s_sb = cpool.tile((1, K), F32)
            if SHARD_W1:
                nc.vector.tensor_add(s_sb[:], sall_sb[:], b2r_sb[:])
            else:
                nc.vector.tensor_add(s_sb[:], psum_s[:], b2r_sb[:])
            smax = cpool.tile((1, 1), F32)
            nc.vector.reduce_max(smax[:], s_sb[:], axis=mybir.AxisListType.X)
            ssub = cpool.tile((1, K), F32)
            nc.vector.tensor_scalar_sub(ssub[:], s_sb[:], smax[:])
            e_sb = cpool.tile((1, K), F32)
            nc.scalar.activation(e_sb[:], ssub[:],
                                 mybir.ActivationFunctionType.Exp)
            esum = cpool.tile((1, 1), F32)
            nc.vector.reduce_sum(esum[:], e_sb[:], axis=mybir.AxisListType.X)
            rinv = cpool.tile((1, 1), F32)
            nc.vector.reciprocal(rinv[:], esum[:])
            alpha_row = cpool.tile((1, K), F32)
            nc.vector.tensor_scalar_mul(alpha_row[:], e_sb[:], rinv[:])

            # broadcast alpha to all 128 partitions: (128, 8)
            psum_ab = ppool.tile((128, K), F32)
            nc.tensor.matmul(psum_ab[:], ones_p[:], alpha_row[:],
                             start=True, stop=True)
            alpha_b = cpool.tile((128, K), F32)
            nc.vector.tensor_copy(alpha_b[:], psum_ab[:])

            # alpha column (8, 1) for bias blend
            psum_ac = ppool.tile((K, 1), F32)
            nc.tensor.matmul(psum_ac[:], alpha_row[:], ones1[:],
                             start=True, stop=True)
            alpha_c = cpool.tile((K, 1), F32)
            nc.vector.tensor_copy(alpha_c[:], psum_ac[:])

            # blended bias row agg_b (1, OC) = alpha_row @ kernels_bias shard
            psum_bb = ppool.tile((1, OC), F32)
            nc.tensor.matmul(psum_bb[:], alpha_c[:], kb_sb[:],
                             start=True, stop=True)
            aggb_sb = cpool.tile((1, OC), BF16)
            nc.vector.tensor_copy(aggb_sb[:], psum_bb[:])

            # xk[:, k, :] = alpha_k * xT (pre-scaled stationaries)
            xk_sb = cpool.tile((128, K, JT * B), F32R_MAIN)
            for k in range(K):
                nc.vector.tensor_scalar_mul(xk_sb[:, k, :], xt_sb[:],
                                            alpha_b[:, k:k + 1])

            # --- main contraction: out (B, OC) in one PSUM group ---
            out_psum = ppool.tile((B, OC), F32)
            first = True
            for (k, j0, nj), slab in zip(SLAB_PLAN, slabs):
                for jl in range(nj):
                    j = j0 + jl
                    nc.tensor.matmul(
                        out_psum[:],
                        xk_sb[:, k, j * B:(j + 1) * B],
                        slab[:, jl * OC:(jl + 1) * OC],
                        start=first, stop=False,
                    )
                    first = False
            # + broadcasted bias row via rank-1 matmul
            nc.tensor.matmul(out_psum[:], ones_b[:], aggb_sb[:],
                             start=False, stop=True)

            y_sb = cpool.tile((B, OC), F32)
            nc.vector.tensor_copy(y_sb[:], out_psum[:])
            nc.scalar.dma_start(y_d.ap(), y_sb[:])
            nc.vector.tensor_copy(dum_sink[:], dum_psum[0:1, 0:1])
            nc.scalar.dma_start(ysink_d.ap(), dum_sink[:])

    nc.compile()
    return nc


def _prep_inputs(x, condition, w1, b1, w2, b2, kernels_weights, kernels_bias):
    """Layout-only host prep: slice per-core shards and retile for DMA."""
    f = np.float32
    x = np.asarray(x, f)
    condition = np.asarray(condition, f)
    w1 = np.asarray(w1, f)
    b1 = np.asarray(b1, f)
    w2 = np.asarray(w2, f)
    b2 = np.asarray(b2, f)
    kernels_weights = np.asarray(kernels_weights, f)
    kernels_bias = np.asarray(kernels_bias, f)

    # xT tiled: xt[p, j*B + b] = x[b, j*128 + p]
    xt = np.ascontiguousarray(
        x.T.reshape(JT, 128, B).transpose(1, 0, 2)).reshape(128, JT * B)
    import ml_dtypes
    bf16 = ml_dtypes.bfloat16
    # condT tiled: ct[p, t] = condition[0, t*128 + p]
    ct = np.ascontiguousarray(condition.reshape(JT, 128).T).astype(bf16)
    # w1 tiled as rhs: w1t[p, t*HL + h] = w1[t*128 + p, hsl(h)]
    w1t_full = np.ascontiguousarray(
        w1.reshape(JT, 128, H).transpose(1, 0, 2))  # (128, JT, H)
    w2t_full = np.ascontiguousarray(
        w2.reshape(HT, 128, K).transpose(1, 0, 2)).reshape(128, HT * K)
    b2r = np.ascontiguousarray(b2.reshape(1, K))

    shard_w1 = os.environ.get("KERNEL_SHARD_W1", "0") == "1"
    in_maps = []
    for c in range(NCORES):
        osl = slice(c * OC, (c + 1) * OC)
        # W shard [k, o, i] -> tiles [k, p, j, o] with i = j*128 + p
        wt = np.ascontiguousarray(
            kernels_weights[:, osl, :].reshape(K, OC, JT, 128)
            .transpose(0, 3, 2, 1)).reshape(K, 128, JT * OC)
        if os.environ.get("KERNEL_WT_DTYPE", "fp16") == "fp16":
            wt = wt.astype(np.float16)
        kb = np.ascontiguousarray(kernels_bias[:, osl])
        if shard_w1:
            hsl = slice(c * HS, (c + 1) * HS)
            w1t = np.ascontiguousarray(
                w1t_full[:, :, hsl]).reshape(128, JT * HS).astype(bf16)
            w2t = np.ascontiguousarray(w2[hsl, :])
            b1r = np.ascontiguousarray(b1[hsl].reshape(1, HS))
        else:
            w1t = w1t_full.reshape(128, JT * H).astype(bf16)
            w2t = w2t_full
            b1r = np.ascontiguousarray(b1.reshape(1, H))
        in_maps.append({
            "wt": wt, "xt": xt, "ct": ct, "w1t": w1t, "w2t": w2t,
            "b1r": b1r, "b2r": b2r, "kb": kb,
        })
    return in_maps


def kernel(x, condition, w1, b1, w2, b2, kernels_weights, kernels_bias):
    global LAST_RESULTS
    key = ("nc", SHARD_W1)
    if key not in _CACHE:
        _CACHE[key] = _build_module()
    nc = _CACHE[key]

    in_maps = _prep_inputs(x, condition, w1, b1, w2, b2,
                           kernels_weights, kernels_bias)

    res = run_bass_kernel_spmd(nc, in_maps, core_ids=list(range(NCORES)))
    LAST_RESULTS = res

    out = np.concatenate([res.results[c]["y"] for c in range(NCORES)], axis=1)
    return np.ascontiguousarray(out, dtype=np.float32)


if __name__ == "__main__":
    # smoke test with random data
    rng = np.random.default_rng(0)
    ins = {
        "x": rng.standard_normal((B, IN), dtype=np.float32),
        "condition": rng.standard_normal((1, IN), dtype=np.float32),
        "w1": rng.standard_normal((IN, H), dtype=np.float32) * 0.02,
        "b1": np.zeros(H, np.float32),
        "w2": rng.standard_normal((H, K), dtype=np.float32) * 0.02,
        "b2": np.zeros(K, np.float32),
        "kernels_weights": rng.standard_normal((K, OUT, IN), dtype=np.float32) * 0.01,
        "kernels_bias": np.zeros((K, OUT), np.float32),
    }
    y = kernel(**ins)
    print("out", y.shape, y.dtype, float(np.abs(y).mean()))


# revision 20
# speedup vs baseline: 1.2048x; 1.2048x over previous
"""Trainium2 Bass kernel for DynamicCondLinear (MoE-routing style).

Math: condition batch is 1, so softmax routing weights alpha (K=8) are shared
by all 32 samples; out = sum_k alpha_k * (x @ W_k^T) + sum_k alpha_k * b_k.

Sharding: tensor-parallel over OUT channels (2048 / 8 cores = 256 per core).
Each core streams its 16 MiB weight shard from HBM once (memory roofline),
computes alpha redundantly (w1 replicated), and does the whole contraction as
accumulating float32r matmuls in a single PSUM tile.

Host-side prep is layout-only (transpose/reshape for DMA-friendly tiling).
"""

import os
import sys

import numpy as np

if "/opt/trn_rl_repo" not in sys.path:
    sys.path.insert(0, "/opt/trn_rl_repo")

import concourse.bacc as bacc
import concourse.mybir as mybir
import concourse.tile as tile
from concourse.bass_utils import run_bass_kernel_spmd

B, IN, OUT, K, H = 32, 2048, 2048, 8, 512
NCORES = 8
OC = OUT // NCORES  # 256 out channels per core
JT = IN // 128      # 16 contraction tiles
HT = H // 128       # 4 hidden tiles
HS = H // NCORES    # 64 hidden units per core when w1 is sharded
# sharding w1 + AllReduce of partial scores measured 190us vs 63us
# replicated: the 8-core collective costs ~120us here, far more than
# the 5us of DMA it saves. Keep w1 replicated (bf16 halves it anyway).
SHARD_W1 = os.environ.get("KERNEL_SHARD_W1", "0") == "1"

F32 = mybir.dt.float32
F32R = mybir.dt.float32r
BF16 = mybir.dt.bfloat16
FP16 = mybir.dt.float16
if os.environ.get("KERNEL_NO_F32R", "0") == "1":
    F32R = mybir.dt.float32  # debug toggle: run everything in plain fp32
# Main path (wt/xk) dtype: fp16 has the same 10-bit mantissa as f32r
# (TF32) -> identical output error (measured 2.95e-4 both ways) at half
# the HBM bytes. f32r/f32 available as fallbacks via env.
_WT = os.environ.get("KERNEL_WT_DTYPE", "fp16")
F32R_MAIN = {"fp16": FP16, "f32r": F32R, "f32": F32}[_WT]

_CACHE = {}

# test.py reads this after calling kernel() to get profiling info
LAST_RESULTS = None


def _build_module():
    """Build the (SPMD-identical) Bass program once."""
    nc = bacc.Bacc("TRN2", target_bir_lowering=False, debug=False,
                   num_devices=NCORES)

    # --- DRAM I/O (per-core data differs, program is identical) ---
    # Pure-matmul operands are declared float32r end-to-end (same bits as
    # fp32; PE runs them at 1 cyc/row instead of 4).
    wt_d = nc.dram_tensor("wt", (K, 128, JT * OC), F32R_MAIN, kind="ExternalInput")
    xt_d = nc.dram_tensor("xt", (128, JT * B), F32, kind="ExternalInput")
    HL = HS if SHARD_W1 else H  # local hidden width
    ct_d = nc.dram_tensor("ct", (128, JT), BF16, kind="ExternalInput")
    w1t_d = nc.dram_tensor("w1t", (128, JT * HL), BF16, kind="ExternalInput")
    if SHARD_W1:
        w2t_d = nc.dram_tensor("w2t", (HS, K), F32, kind="ExternalInput")
    else:
        w2t_d = nc.dram_tensor("w2t", (128, HT * K), F32, kind="ExternalInput")
    b1r_d = nc.dram_tensor("b1r", (1, HL), F32, kind="ExternalInput")
    b2r_d = nc.dram_tensor("b2r", (1, K), F32, kind="ExternalInput")
    kb_d = nc.dram_tensor("kb", (K, OC), F32, kind="ExternalInput")
    y_d = nc.dram_tensor("y", (B, OC), F32, kind="ExternalOutput")
    # warmup sink: consumed so bacc's DCE keeps the PE warm-up matmuls
    ysink_d = nc.dram_tensor("ysink", (1, 1), F32, kind="ExternalOutput")

    with tile.TileContext(nc) as tc:
        with (
            tc.tile_pool(name="cpool", bufs=1) as cpool,
            tc.tile_pool(name="wpool", bufs=16) as wpool,
            tc.tile_pool(name="ppool", bufs=1, space="PSUM") as ppool,
        ):
            # --- weight stream first: 16 resident 1 MiB slabs on the sync
            # HWDGE ring (the critical stream; small loads go on the scalar
            # ring so they don't serialize in front of it) ---
            # alpha-MLP weights go FIRST on the sync ring: they gate the
            # serial alpha chain, ahead of the bulk weight stream
            w1t_sb = cpool.tile((128, JT, HL), BF16)
            nc.sync.dma_start(w1t_sb[:], w1t_d.ap())

            # slab plan: (k, j_start, n_j) -- one slab per expert (1 MiB fp16)
            SLAB_PLAN = [(k, 0, JT) for k in range(K)]
            slabs = []
            for (k, j0, nj) in SLAB_PLAN:
                wt_slab = wpool.tile((128, JT * OC), F32R_MAIN, tag="wt_slab",
                                     bufs=len(SLAB_PLAN))
                nc.sync.dma_start(
                    wt_slab[:, :nj * OC],
                    wt_d.ap()[k][:, j0 * OC:(j0 + nj) * OC])
                slabs.append(wt_slab)

            # --- PE warm-up: dependency-free dummy matmuls keep the HAM
            # clock gate at 2.4 GHz while DMAs land and the alpha chain
            # runs on DVE/ACT. Results go to a scratch PSUM bank. ---
            dum_a = cpool.tile((128, B), BF16)
            nc.gpsimd.memset(dum_a[:], 0.0)
            dum_b = cpool.tile((128, OC), BF16)
            nc.gpsimd.memset(dum_b[:], 0.0)
            dum_psum = ppool.tile((B, OC), F32)

            def warmup(n):
                for _ in range(n):
                    nc.tensor.matmul(dum_psum[:], dum_a[:], dum_b[:],
                                     start=True, stop=True)

            dum_sink = cpool.tile((1, 1), F32)

            # --- constant / small loads ---
            ct_sb = cpool.tile((128, JT), BF16)
            nc.scalar.dma_start(ct_sb[:], ct_d.ap())
            if SHARD_W1:
                w2t_sb = cpool.tile((HS, K), F32)
            else:
                w2t_sb = cpool.tile((128, HT, K), F32)
            nc.scalar.dma_start(w2t_sb[:], w2t_d.ap())
            b1r_sb = cpool.tile((1, HL), F32)
            nc.scalar.dma_start(b1r_sb[:], b1r_d.ap())
            b2r_sb = cpool.tile((1, K), F32)
            nc.scalar.dma_start(b2r_sb[:], b2r_d.ap())
            kb_sb = cpool.tile((K, OC), F32)
            nc.scalar.dma_start(kb_sb[:], kb_d.ap())
            xt_sb = cpool.tile((128, JT * B), F32)
            nc.scalar.dma_start(xt_sb[:], xt_d.ap())

            ones1 = cpool.tile((1, 1), F32)
            nc.gpsimd.memset(ones1[:], 1.0)
            ones_b = cpool.tile((1, B), BF16)
            nc.gpsimd.memset(ones_b[:], 1.0)
            ones_p = cpool.tile((1, 128), F32)
            nc.gpsimd.memset(ones_p[:], 1.0)

            warmup(int(os.environ.get("KERNEL_WARMUP1", "48")))

            # --- alpha MLP: h = relu(cond @ w1 + b1), hidden sharded
            # over cores when SHARD_W1 (each core computes HS=64 units and
            # its partial scores; AllReduce sums the (1,8) partials) ---
            psum_h = ppool.tile((1, HL), F32)
            for t in range(JT):
                nc.tensor.matmul(
                    psum_h[:],
                    ct_sb[:, t:t + 1],
                    w1t_sb[:, t, :],
                    start=(t == 0), stop=(t == JT - 1),
                )
            h_tmp = cpool.tile((1, HL), F32)
            nc.vector.tensor_add(h_tmp[:], psum_h[:], b1r_sb[:])
            h_sb = cpool.tile((1, HL), F32)
            nc.vector.tensor_relu(h_sb[:], h_tmp[:])

            # transpose h (1,HL) -> hT (HL? x 1) via tiny matmuls vs ones
            NQ = (HL + 127) // 128
            QW = min(HL, 128)
            psum_ht = ppool.tile((QW, NQ), F32)
            for q in range(NQ):
                nc.tensor.matmul(
                    psum_ht[:, q:q + 1],
                    h_sb[:, q * QW:(q + 1) * QW],
                    ones1[:],
                    start=True, stop=True,
                )
            ht_sb = cpool.tile((QW, NQ), F32)
            nc.vector.tensor_copy(ht_sb[:], psum_ht[:])

            # scores row (1, 8) = sum_q hT[:,q].T @ w2t rows
            psum_s = ppool.tile((1, K), F32)
            if SHARD_W1:
                nc.tensor.matmul(psum_s[:], ht_sb[:, 0:1], w2t_sb[:],
                                 start=True, stop=True)
            else:
                for q in range(NQ):
                    nc.tensor.matmul(
                        psum_s[:],
                        ht_sb[:, q:q + 1],
                        w2t_sb[:, q, :],
                        start=(q == 0), stop=(q == NQ - 1),
                    )

            if SHARD_W1:
                # AllReduce the (1, 8) partial scores across the 8 cores
                with tc.tile_pool(name="dpool", bufs=1, space="DRAM") as dpool:
                    sc_in = dpool.tile((1, K), F32)
                    sc_out = dpool.tile((1, K), F32)
                    sp_sb = cpool.tile((1, K), F32)
                    nc.vector.tensor_copy(sp_sb[:], psum_s[:])
                    nc.gpsimd.dma_start(sc_in[:], sp_sb[:])
                    nc.gpsimd.collective_compute(
                        "AllReduce",
                        mybir.AluOpType.add,
                        replica_groups=[list(range(NCORES))],
                        ins=[sc_in.opt()],
                        outs=[sc_out.opt()],
                    )
                    sall_sb = cpool.tile((1, K), F32)
                    nc.gpsimd.dma_start(sall_sb[:], sc_out[:])

            warmup(int(os.environ.get("KERNEL_WARMUP2", "16")))

            # softmax over the 8 scores (+ b2)
            s_sb = cpool.tile((1, K), F32)
            if SHARD_W1:
                nc.vector.tensor_add(s_sb[:], sall_sb[:], b2r_sb[:])
            else:
                nc.vector.tensor_add(s_sb[:], psum_s[:], b2r_sb[:])
            smax = cpool.tile((1, 1), F32)
            nc.vector.reduce_max(smax[:], s_sb[:], axis=mybir.AxisListType.X)
            ssub = cpool.tile((1, K), F32)
            nc.vector.tensor_scalar_sub(ssub[:], s_sb[:], smax[:])
            e_sb = cpool.tile((1, K), F32)
            nc.scalar.activation(e_sb[:], ssub[:],
                                 mybir.ActivationFunctionType.Exp)
            esum = cpool.tile((1, 1), F32)
            nc.vector.reduce_sum(esum[:], e_sb[:], axis=mybir.AxisListType.X)
            rinv = cpool.tile((1, 1), F32)
            nc.vector.reciprocal(rinv[:], esum[:])
            alpha_row = cpool.tile((1, K), F32)
            nc.vector.tensor_scalar_mul(alpha_row[:], e_sb[:], rinv[:])

            # broadcast alpha to all 128 partitions: (128, 8)
            psum_ab = ppool.tile((128, K), F32)
            nc.tensor.matmul(psum_ab[:], ones_p[:], alpha_row[:],
                             start=True, stop=True)
            alpha_b = cpool.tile((128, K), F32)
            nc.vector.tensor_copy(alpha_b[:], psum_ab[:])

            # alpha column (8, 1) for bias blend
            psum_ac = ppool.tile((K, 1), F32)
            nc.tensor.matmul(psum_ac[:], alpha_row[:], ones1[:],
                             start=True, stop=True)
            alpha_c = cpool.tile((K, 1), F32)
            nc.vector.tensor_copy(alpha_c[:], psum_ac[:])

            # blended bias row agg_b (1, OC) = alpha_row @ kernels_bias shard
            psum_bb = ppool.tile((1, OC), F32)
            nc.tensor.matmul(psum_bb[:], alpha_c[:], kb_sb[:],
                             start=True, stop=True)
            aggb_sb = cpool.tile((1, OC), BF16)
            nc.vector.tensor_copy(aggb_sb[:], psum_bb[:])

            # xk[:, k, :] = alpha_k * xT (pre-scaled stationaries)
            xk_sb = cpool.tile((128, K, JT * B), F32R_MAIN)
            for k in range(K):
                nc.vector.tensor_scalar_mul(xk_sb[:, k, :], xt_sb[:],
                                            alpha_b[:, k:k + 1])

            # --- main contraction: out (B, OC) in one PSUM group ---
            out_psum = ppool.tile((B, OC), F32)
            first = True
            for (k, j0, nj), slab in zip(SLAB_PLAN, slabs):
                for jl in range(nj):
                    j = j0 + jl
                    nc.tensor.matmul(
                        out_psum[:],
                        xk_sb[:, k, j * B:(j + 1) * B],
                        slab[:, jl * OC:(jl + 1) * OC],
                        start=first, stop=False,
                    )
                    first = False
            # + broadcasted bias row via rank-1 matmul
            nc.tensor.matmul(out_psum[:], ones_b[:], aggb_sb[:],
                             start=False, stop=True)

            y_sb = cpool.tile((B, OC), F32)
            nc.vector.tensor_copy(y_sb[:], out_psum[:])
            nc.scalar.dma_start(y_d.ap(), y_sb[:])
            nc.vector.tensor_copy(dum_sink[:], dum_psum[0:1, 0:1])
            nc.scalar.dma_start(ysink_d.ap(), dum_sink[:])

    nc.compile()
    return nc


def _prep_inputs(x, condition, w1, b1, w2, b2, kernels_weights, kernels_bias):
    """Layout-only host prep: slice per-core shards and retile for DMA."""
    f = np.float32
    x = np.asarray(x, f)
    condition = np.asarray(condition, f)
    w1 = np.asarray(w1, f)
    b1 = np.asarray(b1, f)
    w2 = np.asarray(w2, f)
    b2 = np.asarray(b2, f)
    kernels_weights = np.asarray(kernels_weights, f)
    kernels_bias = np.asarray(kernels_bias, f)

    # xT tiled: xt[p, j*B + b] = x[b, j*128 + p]
    xt = np.ascontiguousarray(
        x.T.reshape(JT, 128, B).transpose(1, 0, 2)).reshape(128, JT * B)
    import ml_dtypes
    bf16 = ml_dtypes.bfloat16
    # condT tiled: ct[p, t] = condition[0, t*128 + p]
    ct = np.ascontiguousarray(condition.reshape(JT, 128).T).astype(bf16)
    # w1 tiled as rhs: w1t[p, t*HL + h] = w1[t*128 + p, hsl(h)]
    w1t_full = np.ascontiguousarray(
        w1.reshape(JT, 128, H).transpose(1, 0, 2))  # (128, JT, H)
    w2t_full = np.ascontiguousarray(
        w2.reshape(HT, 128, K).transpose(1, 0, 2)).reshape(128, HT * K)
    b2r = np.ascontiguousarray(b2.reshape(1, K))

    shard_w1 = os.environ.get("KERNEL_SHARD_W1", "0") == "1"
    in_maps = []
    for c in range(NCORES):
        osl = slice(c * OC, (c + 1) * OC)
        # W shard [k, o, i] -> tiles [k, p, j, o] with i = j*128 + p
        wt = np.ascontiguousarray(
            kernels_weights[:, osl, :].reshape(K, OC, JT, 128)
            .transpose(0, 3, 2, 1)).reshape(K, 128, JT * OC)
        if os.environ.get("KERNEL_WT_DTYPE", "fp16") == "fp16":
            wt = wt.astype(np.float16)
        kb = np.ascontiguousarray(kernels_bias[:, osl])
        if shard_w1:
            hsl = slice(c * HS, (c + 1) * HS)
            w1t = np.ascontiguousarray(
                w1t_full[:, :, hsl]).reshape(128, JT * HS).astype(bf16)
            w2t = np.ascontiguousarray(w2[hsl, :])
            b1r = np.ascontiguousarray(b1[hsl].reshape(1, HS))
        else:
            w1t = w1t_full.reshape(128, JT * H).astype(bf16)
            w2t = w2t_full
            b1r = np.ascontiguousarray(b1.reshape(1, H))
        in_maps.append({
            "wt": wt, "xt": xt, "ct": ct, "w1t": w1t, "w2t": w2t,
            "b1r": b1r, "b2r": b2r, "kb": kb,
        })
    return in_maps


def kernel(x, condition, w1, b1, w2, b2, kernels_weights, kernels_bias):
    global LAST_RESULTS
    key = ("nc", SHARD_W1)
    if key not in _CACHE:
        _CACHE[key] = _build_module()
    nc = _CACHE[key]

    in_maps = _prep_inputs(x, condition, w1, b1, w2, b2,
                           kernels_weights, kernels_bias)

    res = run_bass_kernel_spmd(nc, in_maps, core_ids=list(range(NCORES)))
    LAST_RESULTS = res

    out = np.concatenate([res.results[c]["y"] for c in range(NCORES)], axis=1)
    return np.ascontiguousarray(out, dtype=np.float32)


if __name__ == "__main__":
    # smoke test with random data
    rng = np.random.default_rng(0)
    ins = {
        "x": rng.standard_normal((B, IN), dtype=np.float32),
        "condition": rng.standard_normal((1, IN), dtype=np.float32),
        "w1": rng.standard_normal((IN, H), dtype=np.float32) * 0.02,
        "b1": np.zeros(H, np.float32),
        "w2": rng.standard_normal((H, K), dtype=np.float32) * 0.02,
        "b2": np.zeros(K, np.float32),
        "kernels_weights": rng.standard_normal((K, OUT, IN), dtype=np.float32) * 0.01,
        "kernels_bias": np.zeros((K, OUT), np.float32),
    }
    y = kernel(**ins)
    print("out", y.shape, y.dtype, float(np.abs(y).mean()))
